# revision 49
# baseline (speedup 1.0000x reference)
"""Causal multi-head attention (B=1, S=4096, E=1024, H=16, Dk=64) on 8 TRN2
NeuronCores via Bass/Tile, head-sharded (tensor parallel): core c computes
heads 2c and 2c+1 end-to-end plus its partial output projection; the host sums
the 8 partials and adds the output bias.

Per-core program:
  QT/KT[e'=128, S] = (W x^T + b) in bf16 (softmax 1/sqrt(Dk) folded into Wq/bq)
  V'[k, 2*65]      = x Wv^T + bv, with a ones column per head
  per q-block (512) x k-tile (128, causal):
    scoresT[k, q] via PE (2 heads packed with row tiling, d=64 each)
    pT = exp(scoresT) on ScalarE (no max subtraction; scores are ~N(0,1))
    diagonal tiles: multiply by causal 0/1 mask strip (post-exp)
    acc_h[65, q] += V'_h.T @ pT_h   (row 64 accumulates the softmax denom)
  attn_cT = acc[0:64] * broadcast(1/acc[64]);  partial = attn_cT.T @ Wo_c.T
"""

import numpy as np

import concourse.bass as bass
import concourse.mybir as mybir
import concourse.tile as tile
from concourse import bacc
from concourse.bass_utils import run_bass_kernel_spmd

F32 = mybir.dt.float32
BF16 = mybir.dt.bfloat16
AF = mybir.ActivationFunctionType

EMBED_DIM = 1024
NUM_HEADS = 16
SEQ = 4096
BATCH = 1
N_CORES = 8


def _build_nc(S=SEQ, E=EMBED_DIM):
    EC = 128          # per-core feature slice (2 heads x 64)
    NI = E // 128     # contraction tiles for projections
    NQB = S // 512    # q blocks
    NKT = S // 128    # k tiles

    nc = bacc.Bacc(None, target_bir_lowering=False, debug=False)

    # x arrives pre-permuted to the SBUF layout: xP[p, sb, it, s'] =
    # x[sb*512+s', it*128+p] -- one contiguous 8KB line per partition per
    # 512-column s-block (full-rate DMA, no mid-dim segmentation)
    xP = nc.dram_tensor("xP", [128, S // 512, E // 128, 512], BF16,
                        kind="ExternalInput")
    # projection weights arrive pre-packed as [128, NI*EC]:
    # packed[p, it*EC + e] = W.T[it*128 + p, e]  (contiguous DMA lines)
    wqT = nc.dram_tensor("wqT", [128, NI * EC], BF16, kind="ExternalInput")
    wkT = nc.dram_tensor("wkT", [128, NI * EC], BF16, kind="ExternalInput")
    wvT = nc.dram_tensor("wvT", [128, NI * EC], BF16, kind="ExternalInput")
    woT = nc.dram_tensor("woT", [EC, E], BF16, kind="ExternalInput")
    bq = nc.dram_tensor("bq", [EC, 1], F32, kind="ExternalInput")
    bk = nc.dram_tensor("bk", [EC, 1], F32, kind="ExternalInput")
    bv = nc.dram_tensor("bv", [1, EC], F32, kind="ExternalInput")
    maskst = nc.dram_tensor("maskst", [128, 896], BF16, kind="ExternalInput")
    out = nc.dram_tensor("out", [S, E], F32, kind="ExternalOutput")

    with tile.TileContext(nc) as tc:
        with tc.tile_pool(name="const", bufs=1) as const:
            # q/k weights + small constants first, then x (the long pole),
            # then v/o weights (not needed until attention starts)
            w_sb = {}
            for name, wt in (("q", wqT), ("k", wkT), ("v", wvT)):
                w_sb[name] = const.tile([128, NI, EC], BF16, tag=f"w{name}",
                                        name=f"w{name}")
            for name, wt in (("q", wqT), ("k", wkT)):
                nc.sync.dma_start(
                    out=w_sb[name][:, :, :],
                    in_=wt.ap().rearrange("p (t e) -> p t e", t=NI))

            # x streamed in s-block-major chunks: the first 1 MiB (s-block 0,
            # all 8 i-tiles) lands early so projections/attention start
            # early; later s-blocks stream behind attention demand.
            # s-block-major to mirror xP: contiguous 8KB DMA lines/partition
            xt_sb = const.tile([128, S // 512, NI, 512], BF16, tag="xt")
            bq_sb = const.tile([128, 1], F32, tag="bq")
            bk_sb = const.tile([128, 1], F32, tag="bk")
            bv_row = const.tile([1, EC], F32, tag="bvr")
            bv_bc = const.tile([128, EC], F32, tag="bv")
            mask_sb = const.tile([128, 896], BF16, tag="mask")
            wo_sb = const.tile([128, E], BF16, tag="wo")
            for sb in range(S // 512):
                nc.sync.dma_start(out=xt_sb[:, sb, :, :], in_=xP[:, sb, :, :])
                if sb == 0:
                    nc.sync.dma_start(out=bq_sb, in_=bq[:, :])
                    nc.sync.dma_start(out=bk_sb, in_=bk[:, :])
                    nc.sync.dma_start(out=bv_row, in_=bv[:, :])
                    nc.gpsimd.partition_broadcast(bv_bc[:, :], bv_row[0:1, :])
                    nc.sync.dma_start(out=mask_sb, in_=maskst[:, :])
                    nc.sync.dma_start(
                        out=w_sb["v"][:, :, :],
                        in_=wvT.ap().rearrange("p (t e) -> p t e", t=NI))
                elif sb == 1:
                    nc.sync.dma_start(out=wo_sb, in_=woT[:, :])

            qt_sb = const.tile([128, S], BF16, tag="qt")
            kt_sb = const.tile([128, S], BF16, tag="kt")
            v_sb = const.tile([128, NKT, 130], BF16, tag="v")
            nc.vector.memset(v_sb[:, :, 64:65], 1.0)
            nc.vector.memset(v_sb[:, :, 129:130], 1.0)

            # single PSUM pool; tags shared across phases so banks flow from
            # projections into attention without a phase barrier.
            # banks: sc 2x2 + acc0/acc1 1x1 each + op 2x1 = 8
            with tc.tile_pool(name="ps", bufs=1, space="PSUM") as ps_pool, \
                 tc.tile_pool(name="spt", bufs=8) as spt, \
                 tc.tile_pool(name="sat", bufs=8) as sat, \
                 tc.tile_pool(name="sdiv", bufs=6) as sdiv, \
                 tc.tile_pool(name="sout", bufs=12) as sout:

                def emit_qkproj_one(name, dst, bias, sb):
                    w = w_sb[name]
                    ps = ps_pool.tile([128, 1024], F32, tag="sc", bufs=2,
                                      name=f"pj{name}{sb}")
                    for it in range(NI):
                        nc.tensor.matmul(
                            ps[:, 0:512],
                            lhsT=w[:, it, :],
                            rhs=xt_sb[:, sb, it, :],
                            start=(it == 0), stop=(it == NI - 1),
                        )
                    nc.vector.tensor_scalar_add(
                        dst[:, sb * 512:(sb + 1) * 512], ps[:, 0:512],
                        bias[:, 0:1])

                def emit_qkproj(sb):
                    emit_qkproj_one("q", qt_sb, bq_sb, sb)
                    emit_qkproj_one("k", kt_sb, bk_sb, sb)

                wv = w_sb["v"]
                vproj_done = [0]

                def emit_vproj_one(st):
                    ps = ps_pool.tile([128, 512], F32, tag="op", bufs=2,
                                      name=f"pjv{st}")
                    for it in range(NI):
                        nc.tensor.matmul(
                            ps[:, 0:EC],
                            lhsT=xt_sb[:, st // 4, it,
                                       (st % 4) * 128:(st % 4) * 128 + 128],
                            rhs=wv[:, it, :],
                            start=(it == 0), stop=(it == NI - 1),
                        )
                    nc.vector.tensor_add(
                        v_sb[:, st, 0:64], ps[:, 0:64], bv_bc[:, 0:64])
                    nc.vector.tensor_add(
                        v_sb[:, st, 65:129], ps[:, 64:128], bv_bc[:, 64:128])

                def emit_vproj(upto):
                    for st in range(vproj_done[0], min(upto, NKT)):
                        emit_vproj_one(st)
                    vproj_done[0] = max(vproj_done[0], min(upto, NKT))

                def emit_attnv(acc, jpt, nkt):
                    j, pt, off, w = jpt
                    for h in range(2):
                        nc.tensor.matmul(
                            acc[h][:, off:512],
                            lhsT=v_sb[:, j, 65 * h:65 * h + 65],
                            rhs=pt[:, 512 * h:512 * h + w],
                            start=(j == 0), stop=(j == nkt - 1),
                        )

                pending_oproj = []

                # HAM warmup: run throwaway matmuls while the first DMAs are
                # in flight so the real projections start at the warm clock.
                warm_src = const.tile([128, 512], BF16, tag="warmsrc")
                nc.vector.memset(warm_src[:, :], 1.0)
                for i in range(14):
                    wp = ps_pool.tile([128, 1024], F32, tag="sc", bufs=2,
                                      name=f"warm{i}")
                    nc.tensor.matmul(wp[:, 0:512], lhsT=warm_src[:, 0:128],
                                     rhs=warm_src[:, :], start=True, stop=True)

                emit_qkproj(0)
                emit_vproj(4)
                for qb in range(NQB):
                    # lookahead projections for qb+1, spread through the k-loop
                    # so they soak PE slack instead of stalling the exp stream
                    bg = []
                    if qb + 1 < NQB:
                        for name, dst, bias in (("q", qt_sb, bq_sb),
                                                ("k", kt_sb, bk_sb)):
                            bg.append(lambda n=name, d=dst, b=bias, s=qb + 1:
                                      emit_qkproj_one(n, d, b, s))
                    lo, hi = vproj_done[0], min(4 * (qb + 2), NKT)
                    for st in range(lo, hi):
                        bg.append(lambda st=st: emit_vproj_one(st))
                    vproj_done[0] = hi

                    nkt = 4 * (qb + 1)
                    acc = [ps_pool.tile([65, 512], F32, tag=f"acc{h}",
                                        name=f"acc{h}_{qb}")
                           for h in range(2)]
                    pts = []
                    for j in range(nkt):
                        if j == 4 and pending_oproj:
                            # previous q-block's output projection, deferred so
                            # the next scores/exp stream starts without a gap
                            pending_oproj.pop()()
                        if j >= 2 and j != 4 and bg:
                            bg.pop(0)()
                        r = j - 4 * qb  # >= 0 on the causal diagonal
                        off = 128 * r if r > 0 else 0
                        w = 512 - off   # valid q columns for this k-tile
                        sc = ps_pool.tile([128, 1024], F32, tag="sc", bufs=2,
                                          name=f"sc{qb}_{j}")
                        for h in range(2):
                            hp = slice(64 * h, 64 * h + 64)
                            nc.tensor.matmul(
                                sc[:, 512 * h:512 * h + w],
                                lhsT=kt_sb[hp, j * 128:(j + 1) * 128],
                                rhs=qt_sb[hp, qb * 512 + off:(qb + 1) * 512],
                                start=True, stop=True,
                            )
                        pt = spt.tile([128, 1024], BF16, tag="pt", name=f"pt{qb}_{j}")
                        if r >= 0:
                            # one exp over both heads' [0:w] and [512:512+w]
                            # slices via a strided AP (keeps PSUM banks aligned)
                            def _two(t, w=w):
                                a = t[:, :]
                                return bass.AP(tensor=a.tensor, offset=a.offset,
                                               ap=[a.ap[0], [512, 2], [1, w]])
                            nc.scalar.activation(_two(pt), _two(sc), AF.Exp)
                            m = mask_sb[:, 384:384 + w]
                            for h in range(2):
                                pslc = pt[:, 512 * h:512 * h + w]
                                nc.vector.tensor_mul(pslc, pslc, m)
                        else:
                            nc.scalar.activation(pt[:, :], sc[:, :], AF.Exp)
                        pts.append((j, pt, off, w))
                        if len(pts) >= 5:
                            emit_attnv(acc, pts.pop(0), nkt)
                    while bg:  # anything not soaked up mid-loop
                        bg.pop(0)()
                    while pts:
                        emit_attnv(acc, pts.pop(0), nkt)

                    att = sat.tile([128, 512], BF16, tag="att", name=f"att{qb}")
                    for h in range(2):
                        rc = sdiv.tile([1, 512], F32, tag=f"rc{h}", name=f"rc{h}_{qb}")
                        nc.vector.reciprocal(rc[:, :], acc[h][64:65, :])
                        rbc = sdiv.tile([64, 512], F32, tag=f"rbc{h}",
                                        name=f"rbc{h}_{qb}")
                        nc.gpsimd.partition_broadcast(rbc[:, :], rc[0:1, :])
                        nc.vector.tensor_mul(
                            att[64 * h:64 * h + 64, :], acc[h][0:64, :], rbc[:, :])

                    last = qb == NQB - 1

                    def emit_oproj(qb=qb, att=att, last=last):
                        for st in range(4):
                            for nh in range(E // 512):
                                op = ps_pool.tile([128, 512], F32, tag="op", bufs=2,
                                                  name=f"op{qb}_{st}_{nh}")
                                nc.tensor.matmul(
                                    op[:, :],
                                    lhsT=att[:, st * 128:(st + 1) * 128],
                                    rhs=wo_sb[:, nh * 512:(nh + 1) * 512],
                                    start=True, stop=True,
                                )
                                ob = sout.tile([128, 512], F32, tag="ob",
                                               name=f"ob{qb}_{st}_{nh}")
                                if last and (st * 2 + nh) % 2 == 0:
                                    # ScalarE is idle in the kernel tail;
                                    # split the PSUM drain across both engines
                                    nc.scalar.copy(ob[:, :], op[:, :])
                                else:
                                    nc.vector.tensor_copy(ob[:, :], op[:, :])
                                nc.sync.dma_start(
                                    out=out[qb * 512 + st * 128:
                                            qb * 512 + (st + 1) * 128,
                                            nh * 512:(nh + 1) * 512],
                                    in_=ob[:, :])

                    pending_oproj.append(emit_oproj)

                while pending_oproj:
                    pending_oproj.pop()()

    nc.compile()
    return nc


def _make_mask_strip():
    k = np.arange(128)[:, None]
    t = np.arange(896)[None, :]
    return (k <= t - 384).astype(np.float32)


def _pack_w(wT):
    # [E, EC] -> [128, NI*EC] with packed[p, it*EC+e] = wT[it*128+p, e]
    E, EC = wT.shape
    return np.ascontiguousarray(
        wT.reshape(E // 128, 128, EC).transpose(1, 0, 2).reshape(128, -1))


def _shard_inputs(x, Wq, bq, Wk, bk, Wv, bv, Wo):
    import ml_dtypes
    bf16 = ml_dtypes.bfloat16
    S, E = x.shape[-2], x.shape[-1]
    xP = np.ascontiguousarray(
        np.asarray(x, np.float32).reshape(S // 512, 512, E // 128, 128)
        .transpose(3, 0, 2, 1)).astype(bf16)
    strip = _make_mask_strip().astype(bf16)
    in_maps = []
    for c in range(N_CORES):
        sl = slice(128 * c, 128 * (c + 1))
        in_maps.append({
            "xP": xP,
            "wqT": _pack_w((np.asarray(Wq, np.float32)[sl, :] / 8.0).T).astype(bf16),
            "wkT": _pack_w(np.asarray(Wk, np.float32)[sl, :].T).astype(bf16),
            "wvT": _pack_w(np.asarray(Wv, np.float32)[sl, :].T).astype(bf16),
            "woT": np.ascontiguousarray(np.asarray(Wo, np.float32)[:, sl].T).astype(bf16),
            "bq": (np.asarray(bq, np.float32)[sl] / 8.0).reshape(128, 1),
            "bk": np.asarray(bk, np.float32)[sl].reshape(128, 1),
            "bv": np.asarray(bv, np.float32)[sl].reshape(1, 128),
            "maskst": strip,
        })
    return in_maps


_NC_CACHE = {}


def kernel(x, Wq, bq, Wk, bk, Wv, bv, Wo, bo):
    x = np.asarray(x)
    B, S, E = x.shape
    if (S, E) not in _NC_CACHE:
        _NC_CACHE[(S, E)] = _build_nc(S=S, E=E)
    nc = _NC_CACHE[(S, E)]

    in_maps = _shard_inputs(x, Wq, bq, Wk, bk, Wv, bv, Wo)
    res = run_bass_kernel_spmd(nc, in_maps, list(range(N_CORES)))

    total = np.zeros((S, E), np.float32)
    for r in res.results:
        total += r["out"]
    total += np.asarray(bo, np.float32)
    return total.reshape(B, S, E).astype(np.float32)


# revision 58
# speedup vs baseline: 1.0239x; 1.0239x over previous
"""Causal multi-head attention (B=1, S=4096, E=1024, H=16, Dk=64) on 8 TRN2
NeuronCores via Bass/Tile, head-sharded (tensor parallel): core c computes
heads 2c and 2c+1 end-to-end plus its partial output projection; the host sums
the 8 partials and adds the output bias.

Per-core program:
  QT/KT[e'=128, S] = (W x^T + b) in bf16 (softmax 1/sqrt(Dk) folded into Wq/bq)
  V'[k, 2*65]      = x Wv^T + bv, with a ones column per head
  per q-block (512) x k-tile (128, causal):
    scoresT[k, q] via PE (2 heads packed with row tiling, d=64 each)
    pT = exp(scoresT) on ScalarE (no max subtraction; scores are ~N(0,1))
    diagonal tiles: multiply by causal 0/1 mask strip (post-exp)
    acc_h[65, q] += V'_h.T @ pT_h   (row 64 accumulates the softmax denom)
  attn_cT = acc[0:64] * broadcast(1/acc[64]);  partial = attn_cT.T @ Wo_c.T
"""

import numpy as np

import concourse.bass as bass
import concourse.mybir as mybir
import concourse.tile as tile
from concourse import bacc
from concourse.bass_utils import run_bass_kernel_spmd

F32 = mybir.dt.float32
BF16 = mybir.dt.bfloat16
AF = mybir.ActivationFunctionType

EMBED_DIM = 1024
NUM_HEADS = 16
SEQ = 4096
BATCH = 1
N_CORES = 8


def _build_nc(S=SEQ, E=EMBED_DIM):
    EC = 128          # per-core feature slice (2 heads x 64)
    NI = E // 128     # contraction tiles for projections
    NQB = S // 512    # q blocks
    NKT = S // 128    # k tiles

    nc = bacc.Bacc(None, target_bir_lowering=False, debug=False)

    # x arrives pre-permuted to the SBUF layout: xP[p, sb, it, s'] =
    # x[sb*512+s', it*128+p] -- one contiguous 8KB line per partition per
    # 512-column s-block (full-rate DMA, no mid-dim segmentation)
    xP = nc.dram_tensor("xP", [128, S // 512, E // 128, 512], BF16,
                        kind="ExternalInput")
    # projection weights arrive pre-packed as [128, NI*EC]:
    # packed[p, it*EC + e] = W.T[it*128 + p, e]  (contiguous DMA lines)
    wqT = nc.dram_tensor("wqT", [128, NI * EC], BF16, kind="ExternalInput")
    wkT = nc.dram_tensor("wkT", [128, NI * EC], BF16, kind="ExternalInput")
    wvT = nc.dram_tensor("wvT", [128, NI * EC], BF16, kind="ExternalInput")
    woT = nc.dram_tensor("woT", [EC, E], BF16, kind="ExternalInput")
    bq = nc.dram_tensor("bq", [EC, 1], F32, kind="ExternalInput")
    bk = nc.dram_tensor("bk", [EC, 1], F32, kind="ExternalInput")
    bv = nc.dram_tensor("bv", [1, EC], F32, kind="ExternalInput")
    maskst = nc.dram_tensor("maskst", [128, 896], BF16, kind="ExternalInput")
    out = nc.dram_tensor("out", [S, E], F32, kind="ExternalOutput")

    with tile.TileContext(nc) as tc:
        with tc.tile_pool(name="const", bufs=1) as const:
            # q/k weights + small constants first, then x (the long pole),
            # then v/o weights (not needed until attention starts)
            w_sb = {}
            for name, wt in (("q", wqT), ("k", wkT), ("v", wvT)):
                w_sb[name] = const.tile([128, NI, EC], BF16, tag=f"w{name}",
                                        name=f"w{name}")
            for name, wt in (("q", wqT), ("k", wkT)):
                nc.sync.dma_start(
                    out=w_sb[name][:, :, :],
                    in_=wt.ap().rearrange("p (t e) -> p t e", t=NI))

            # x streamed in s-block-major chunks: the first 1 MiB (s-block 0,
            # all 8 i-tiles) lands early so projections/attention start
            # early; later s-blocks stream behind attention demand.
            # s-block-major to mirror xP: contiguous 8KB DMA lines/partition
            xt_sb = const.tile([128, S // 512, NI, 512], BF16, tag="xt")
            bq_sb = const.tile([128, 1], F32, tag="bq")
            bk_sb = const.tile([128, 1], F32, tag="bk")
            bv_row = const.tile([1, EC], F32, tag="bvr")
            bv_bc = const.tile([128, EC], F32, tag="bv")
            mask_sb = const.tile([128, 896], BF16, tag="mask")
            wo_sb = const.tile([128, E], BF16, tag="wo")
            for sb in range(S // 512):
                nc.sync.dma_start(out=xt_sb[:, sb, :, :], in_=xP[:, sb, :, :])
                if sb == 0:
                    nc.sync.dma_start(out=bq_sb, in_=bq[:, :])
                    nc.sync.dma_start(out=bk_sb, in_=bk[:, :])
                    nc.sync.dma_start(out=bv_row, in_=bv[:, :])
                    nc.gpsimd.partition_broadcast(bv_bc[:, :], bv_row[0:1, :])
                    nc.sync.dma_start(out=mask_sb, in_=maskst[:, :])
                    nc.sync.dma_start(
                        out=w_sb["v"][:, :, :],
                        in_=wvT.ap().rearrange("p (t e) -> p t e", t=NI))
                elif sb == 1:
                    nc.sync.dma_start(out=wo_sb, in_=woT[:, :])

            qt_sb = const.tile([128, S], BF16, tag="qt")
            kt_sb = const.tile([128, S], BF16, tag="kt")
            v_sb = const.tile([128, NKT, 130], BF16, tag="v")
            nc.vector.memset(v_sb[:, :, 64:65], 1.0)
            nc.vector.memset(v_sb[:, :, 129:130], 1.0)

            # single PSUM pool; tags shared across phases so banks flow from
            # projections into attention without a phase barrier.
            # banks: sc 2x2 + acc0/acc1 1x1 each + op 2x1 = 8
            with tc.tile_pool(name="ps", bufs=1, space="PSUM") as ps_pool, \
                 tc.tile_pool(name="spt", bufs=10) as spt, \
                 tc.tile_pool(name="sat", bufs=8) as sat, \
                 tc.tile_pool(name="sdiv", bufs=6) as sdiv, \
                 tc.tile_pool(name="sout", bufs=12) as sout:

                def emit_qkproj_one(name, dst, bias, sb):
                    w = w_sb[name]
                    ps = ps_pool.tile([128, 1024], F32, tag="sc", bufs=2,
                                      name=f"pj{name}{sb}")
                    for it in range(NI):
                        nc.tensor.matmul(
                            ps[:, 0:512],
                            lhsT=w[:, it, :],
                            rhs=xt_sb[:, sb, it, :],
                            start=(it == 0), stop=(it == NI - 1),
                        )
                    nc.vector.tensor_scalar_add(
                        dst[:, sb * 512:(sb + 1) * 512], ps[:, 0:512],
                        bias[:, 0:1])

                def emit_qkproj(sb):
                    emit_qkproj_one("q", qt_sb, bq_sb, sb)
                    emit_qkproj_one("k", kt_sb, bk_sb, sb)

                wv = w_sb["v"]
                vproj_done = [0]

                def emit_vproj_one(st):
                    ps = ps_pool.tile([128, 512], F32, tag="op", bufs=2,
                                      name=f"pjv{st}")
                    for it in range(NI):
                        nc.tensor.matmul(
                            ps[:, 0:EC],
                            lhsT=xt_sb[:, st // 4, it,
                                       (st % 4) * 128:(st % 4) * 128 + 128],
                            rhs=wv[:, it, :],
                            start=(it == 0), stop=(it == NI - 1),
                        )
                    nc.vector.tensor_add(
                        v_sb[:, st, 0:64], ps[:, 0:64], bv_bc[:, 0:64])
                    nc.vector.tensor_add(
                        v_sb[:, st, 65:129], ps[:, 64:128], bv_bc[:, 64:128])

                def emit_vproj(upto):
                    for st in range(vproj_done[0], min(upto, NKT)):
                        emit_vproj_one(st)
                    vproj_done[0] = max(vproj_done[0], min(upto, NKT))

                def emit_attnv(acc, jpt, nkt):
                    j, pt, off, w = jpt
                    for h in range(2):
                        nc.tensor.matmul(
                            acc[h][:, off:512],
                            lhsT=v_sb[:, j, 65 * h:65 * h + 65],
                            rhs=pt[:, 512 * h:512 * h + w],
                            start=(j == 0), stop=(j == nkt - 1),
                        )

                pending_oproj = []

                # HAM warmup: run throwaway matmuls while the first DMAs are
                # in flight so the real projections start at the warm clock.
                warm_src = const.tile([128, 512], BF16, tag="warmsrc")
                nc.vector.memset(warm_src[:, :], 1.0)
                for i in range(14):
                    wp = ps_pool.tile([128, 1024], F32, tag="sc", bufs=2,
                                      name=f"warm{i}")
                    nc.tensor.matmul(wp[:, 0:512], lhsT=warm_src[:, 0:128],
                                     rhs=warm_src[:, :], start=True, stop=True)

                emit_qkproj(0)
                emit_vproj(4)
                for qb in range(NQB):
                    # lookahead projections for qb+1, spread through the k-loop
                    # so they soak PE slack instead of stalling the exp stream
                    bg = []
                    if qb + 1 < NQB:
                        for name, dst, bias in (("q", qt_sb, bq_sb),
                                                ("k", kt_sb, bk_sb)):
                            bg.append(lambda n=name, d=dst, b=bias, s=qb + 1:
                                      emit_qkproj_one(n, d, b, s))
                    lo, hi = vproj_done[0], min(4 * (qb + 2), NKT)
                    for st in range(lo, hi):
                        bg.append(lambda st=st: emit_vproj_one(st))
                    vproj_done[0] = hi

                    nkt = 4 * (qb + 1)
                    acc = [ps_pool.tile([65, 512], F32, tag=f"acc{h}",
                                        name=f"acc{h}_{qb}")
                           for h in range(2)]
                    pts = []
                    for j in range(nkt):
                        if j == 6 and pending_oproj:
                            # previous q-block's output projection, deferred so
                            # the next scores/exp stream starts without a gap
                            pending_oproj.pop()()
                        if j >= 2 and j != 6 and bg:
                            bg.pop(0)()
                        r = j - 4 * qb  # >= 0 on the causal diagonal
                        off = 128 * r if r > 0 else 0
                        w = 512 - off   # valid q columns for this k-tile
                        sc = ps_pool.tile([128, 1024], F32, tag="sc", bufs=2,
                                          name=f"sc{qb}_{j}")
                        for h in range(2):
                            hp = slice(64 * h, 64 * h + 64)
                            nc.tensor.matmul(
                                sc[:, 512 * h:512 * h + w],
                                lhsT=kt_sb[hp, j * 128:(j + 1) * 128],
                                rhs=qt_sb[hp, qb * 512 + off:(qb + 1) * 512],
                                start=True, stop=True,
                            )
                        pt = spt.tile([128, 1024], BF16, tag="pt", name=f"pt{qb}_{j}")
                        if r >= 0:
                            # one exp over both heads' [0:w] and [512:512+w]
                            # slices via a strided AP (keeps PSUM banks aligned)
                            def _two(t, w=w):
                                a = t[:, :]
                                return bass.AP(tensor=a.tensor, offset=a.offset,
                                               ap=[a.ap[0], [512, 2], [1, w]])
                            nc.scalar.activation(_two(pt), _two(sc), AF.Exp)
                            m = mask_sb[:, 384:384 + w]
                            for h in range(2):
                                pslc = pt[:, 512 * h:512 * h + w]
                                nc.vector.tensor_mul(pslc, pslc, m)
                        else:
                            nc.scalar.activation(pt[:, :], sc[:, :], AF.Exp)
                        pts.append((j, pt, off, w))
                        if len(pts) >= 7:
                            emit_attnv(acc, pts.pop(0), nkt)
                    while bg:  # anything not soaked up mid-loop
                        bg.pop(0)()
                    while pts:
                        emit_attnv(acc, pts.pop(0), nkt)

                    att = sat.tile([128, 512], BF16, tag="att", name=f"att{qb}")
                    for h in range(2):
                        rc = sdiv.tile([1, 512], F32, tag=f"rc{h}", name=f"rc{h}_{qb}")
                        nc.vector.reciprocal(rc[:, :], acc[h][64:65, :])
                        rbc = sdiv.tile([64, 512], F32, tag=f"rbc{h}",
                                        name=f"rbc{h}_{qb}")
                        nc.gpsimd.partition_broadcast(rbc[:, :], rc[0:1, :])
                        nc.vector.tensor_mul(
                            att[64 * h:64 * h + 64, :], acc[h][0:64, :], rbc[:, :])

                    last = qb == NQB - 1

                    def emit_oproj(qb=qb, att=att, last=last):
                        for st in range(4):
                            for nh in range(E // 512):
                                op = ps_pool.tile([128, 512], F32, tag="op", bufs=2,
                                                  name=f"op{qb}_{st}_{nh}")
                                nc.tensor.matmul(
                                    op[:, :],
                                    lhsT=att[:, st * 128:(st + 1) * 128],
                                    rhs=wo_sb[:, nh * 512:(nh + 1) * 512],
                                    start=True, stop=True,
                                )
                                ob = sout.tile([128, 512], F32, tag="ob",
                                               name=f"ob{qb}_{st}_{nh}")
                                if last and (st * 2 + nh) % 2 == 0:
                                    # ScalarE is idle in the kernel tail;
                                    # split the PSUM drain across both engines
                                    nc.scalar.copy(ob[:, :], op[:, :])
                                else:
                                    nc.vector.tensor_copy(ob[:, :], op[:, :])
                                nc.sync.dma_start(
                                    out=out[qb * 512 + st * 128:
                                            qb * 512 + (st + 1) * 128,
                                            nh * 512:(nh + 1) * 512],
                                    in_=ob[:, :])

                    pending_oproj.append(emit_oproj)

                while pending_oproj:
                    pending_oproj.pop()()

    nc.compile()
    return nc


def _make_mask_strip():
    k = np.arange(128)[:, None]
    t = np.arange(896)[None, :]
    return (k <= t - 384).astype(np.float32)


def _pack_w(wT):
    # [E, EC] -> [128, NI*EC] with packed[p, it*EC+e] = wT[it*128+p, e]
    E, EC = wT.shape
    return np.ascontiguousarray(
        wT.reshape(E // 128, 128, EC).transpose(1, 0, 2).reshape(128, -1))


def _shard_inputs(x, Wq, bq, Wk, bk, Wv, bv, Wo):
    import ml_dtypes
    bf16 = ml_dtypes.bfloat16
    S, E = x.shape[-2], x.shape[-1]
    xP = np.ascontiguousarray(
        np.asarray(x, np.float32).reshape(S // 512, 512, E // 128, 128)
        .transpose(3, 0, 2, 1)).astype(bf16)
    strip = _make_mask_strip().astype(bf16)
    in_maps = []
    for c in range(N_CORES):
        sl = slice(128 * c, 128 * (c + 1))
        in_maps.append({
            "xP": xP,
            "wqT": _pack_w((np.asarray(Wq, np.float32)[sl, :] / 8.0).T).astype(bf16),
            "wkT": _pack_w(np.asarray(Wk, np.float32)[sl, :].T).astype(bf16),
            "wvT": _pack_w(np.asarray(Wv, np.float32)[sl, :].T).astype(bf16),
            "woT": np.ascontiguousarray(np.asarray(Wo, np.float32)[:, sl].T).astype(bf16),
            "bq": (np.asarray(bq, np.float32)[sl] / 8.0).reshape(128, 1),
            "bk": np.asarray(bk, np.float32)[sl].reshape(128, 1),
            "bv": np.asarray(bv, np.float32)[sl].reshape(1, 128),
            "maskst": strip,
        })
    return in_maps


_NC_CACHE = {}


def kernel(x, Wq, bq, Wk, bk, Wv, bv, Wo, bo):
    x = np.asarray(x)
    B, S, E = x.shape
    if (S, E) not in _NC_CACHE:
        _NC_CACHE[(S, E)] = _build_nc(S=S, E=E)
    nc = _NC_CACHE[(S, E)]

    in_maps = _shard_inputs(x, Wq, bq, Wk, bk, Wv, bv, Wo)
    res = run_bass_kernel_spmd(nc, in_maps, list(range(N_CORES)))

    total = np.zeros((S, E), np.float32)
    for r in res.results:
        total += r["out"]
    total += np.asarray(bo, np.float32)
    return total.reshape(B, S, E).astype(np.float32)


# revision 70
# speedup vs baseline: 1.0435x; 1.0192x over previous
"""Causal multi-head attention (B=1, S=4096, E=1024, H=16, Dk=64) on 8 TRN2
NeuronCores via Bass/Tile, head-sharded (tensor parallel): core c computes
heads 2c and 2c+1 end-to-end plus its partial output projection; the host sums
the 8 partials and adds the output bias.

Per-core program:
  QT/KT[e'=128, S] = (W x^T + b) in bf16 (softmax 1/sqrt(Dk) folded into Wq/bq)
  V'[k, 2*65]      = x Wv^T + bv, with a ones column per head
  per q-block (512) x k-tile (128, causal):
    scoresT[k, q] via PE (2 heads packed with row tiling, d=64 each)
    pT = exp(scoresT) on ScalarE (no max subtraction; scores are ~N(0,1))
    diagonal tiles: multiply by causal 0/1 mask strip (post-exp)
    acc_h[65, q] += V'_h.T @ pT_h   (row 64 accumulates the softmax denom)
  attn_cT = acc[0:64] * broadcast(1/acc[64]);  partial = attn_cT.T @ Wo_c.T
"""

import numpy as np

import concourse.bass as bass
import concourse.mybir as mybir
import concourse.tile as tile
from concourse import bacc
from concourse.bass_utils import run_bass_kernel_spmd

F32 = mybir.dt.float32
BF16 = mybir.dt.bfloat16
AF = mybir.ActivationFunctionType

EMBED_DIM = 1024
NUM_HEADS = 16
SEQ = 4096
BATCH = 1
N_CORES = 8


def _build_nc(S=SEQ, E=EMBED_DIM):
    EC = 128          # per-core feature slice (2 heads x 64)
    NI = E // 128     # contraction tiles for projections
    NQB = S // 512    # q blocks
    NKT = S // 128    # k tiles

    nc = bacc.Bacc(None, target_bir_lowering=False, debug=False)

    # x arrives pre-permuted to the SBUF layout: xP[p, sb, it, s'] =
    # x[sb*512+s', it*128+p] -- one contiguous 8KB line per partition per
    # 512-column s-block (full-rate DMA, no mid-dim segmentation)
    xP = nc.dram_tensor("xP", [128, S // 512, E // 128, 512], BF16,
                        kind="ExternalInput")
    # projection weights arrive pre-packed as [128, NI*EC]:
    # packed[p, it*EC + e] = W.T[it*128 + p, e]  (contiguous DMA lines)
    wqT = nc.dram_tensor("wqT", [128, NI * EC], BF16, kind="ExternalInput")
    wkT = nc.dram_tensor("wkT", [128, NI * EC], BF16, kind="ExternalInput")
    wvT = nc.dram_tensor("wvT", [128, NI * EC], BF16, kind="ExternalInput")
    woT = nc.dram_tensor("woT", [EC, E], BF16, kind="ExternalInput")
    bq = nc.dram_tensor("bq", [EC, 1], F32, kind="ExternalInput")
    bk = nc.dram_tensor("bk", [EC, 1], F32, kind="ExternalInput")
    bv = nc.dram_tensor("bv", [1, EC], F32, kind="ExternalInput")
    maskst = nc.dram_tensor("maskst", [128, 896], BF16, kind="ExternalInput")
    out = nc.dram_tensor("out", [S, E], F32, kind="ExternalOutput")

    with tile.TileContext(nc) as tc:
        with tc.tile_pool(name="const", bufs=1) as const:
            # q/k weights + small constants first, then x (the long pole),
            # then v/o weights (not needed until attention starts)
            w_sb = {}
            for name, wt in (("q", wqT), ("k", wkT), ("v", wvT)):
                w_sb[name] = const.tile([128, NI, EC], BF16, tag=f"w{name}",
                                        name=f"w{name}")
            for name, wt in (("q", wqT), ("k", wkT)):
                nc.sync.dma_start(
                    out=w_sb[name][:, :, :],
                    in_=wt.ap().rearrange("p (t e) -> p t e", t=NI))

            # x streamed in s-block-major chunks: the first 1 MiB (s-block 0,
            # all 8 i-tiles) lands early so projections/attention start
            # early; later s-blocks stream behind attention demand.
            # s-block-major to mirror xP: contiguous 8KB DMA lines/partition
            xt_sb = const.tile([128, S // 512, NI, 512], BF16, tag="xt")
            bq_sb = const.tile([128, 1], F32, tag="bq")
            bk_sb = const.tile([128, 1], F32, tag="bk")
            bv_row = const.tile([1, EC], F32, tag="bvr")
            bv_bc = const.tile([128, EC], F32, tag="bv")
            mask_sb = const.tile([128, 896], BF16, tag="mask")
            wo_sb = const.tile([128, E], BF16, tag="wo")
            for sb in range(S // 512):
                nc.sync.dma_start(out=xt_sb[:, sb, :, :], in_=xP[:, sb, :, :])
                if sb == 0:
                    nc.sync.dma_start(out=bq_sb, in_=bq[:, :])
                    nc.sync.dma_start(out=bk_sb, in_=bk[:, :])
                    nc.sync.dma_start(out=bv_row, in_=bv[:, :])
                    nc.gpsimd.partition_broadcast(bv_bc[:, :], bv_row[0:1, :])
                    nc.sync.dma_start(out=mask_sb, in_=maskst[:, :])
                    nc.sync.dma_start(
                        out=w_sb["v"][:, :, :],
                        in_=wvT.ap().rearrange("p (t e) -> p t e", t=NI))
                elif sb == 1:
                    nc.sync.dma_start(out=wo_sb, in_=woT[:, :])

            qt_sb = const.tile([128, S], BF16, tag="qt")
            kt_sb = const.tile([128, S], BF16, tag="kt")
            v_sb = const.tile([128, NKT, 130], BF16, tag="v")
            nc.vector.memset(v_sb[:, :, 64:65], 1.0)
            nc.vector.memset(v_sb[:, :, 129:130], 1.0)

            # single PSUM pool; tags shared across phases so banks flow from
            # projections into attention without a phase barrier.
            # banks: sc 2x2 + acc0/acc1 1x1 each + op 2x1 = 8
            with tc.tile_pool(name="ps", bufs=1, space="PSUM") as ps_pool, \
                 tc.tile_pool(name="spt", bufs=10) as spt, \
                 tc.tile_pool(name="sat", bufs=9) as sat, \
                 tc.tile_pool(name="sdiv", bufs=6) as sdiv, \
                 tc.tile_pool(name="sout", bufs=12) as sout:

                def emit_qkproj_one(name, dst, bias, sb):
                    w = w_sb[name]
                    ps = ps_pool.tile([128, 1024], F32, tag="sc", bufs=2,
                                      name=f"pj{name}{sb}")
                    for it in range(NI):
                        nc.tensor.matmul(
                            ps[:, 0:512],
                            lhsT=w[:, it, :],
                            rhs=xt_sb[:, sb, it, :],
                            start=(it == 0), stop=(it == NI - 1),
                        )
                    nc.vector.tensor_scalar_add(
                        dst[:, sb * 512:(sb + 1) * 512], ps[:, 0:512],
                        bias[:, 0:1])

                def emit_qkproj(sb):
                    emit_qkproj_one("q", qt_sb, bq_sb, sb)
                    emit_qkproj_one("k", kt_sb, bk_sb, sb)

                wv = w_sb["v"]
                vproj_done = [0]

                def emit_vproj_one(st):
                    ps = ps_pool.tile([128, 512], F32, tag="op", bufs=2,
                                      name=f"pjv{st}")
                    for it in range(NI):
                        nc.tensor.matmul(
                            ps[:, 0:EC],
                            lhsT=xt_sb[:, st // 4, it,
                                       (st % 4) * 128:(st % 4) * 128 + 128],
                            rhs=wv[:, it, :],
                            start=(it == 0), stop=(it == NI - 1),
                        )
                    nc.vector.tensor_add(
                        v_sb[:, st, 0:64], ps[:, 0:64], bv_bc[:, 0:64])
                    nc.vector.tensor_add(
                        v_sb[:, st, 65:129], ps[:, 64:128], bv_bc[:, 64:128])

                def emit_vproj(upto):
                    for st in range(vproj_done[0], min(upto, NKT)):
                        emit_vproj_one(st)
                    vproj_done[0] = max(vproj_done[0], min(upto, NKT))

                def emit_attnv(acc, jpt, nkt):
                    j, pt, off, w = jpt
                    for h in range(2):
                        nc.tensor.matmul(
                            acc[h][:, off:512],
                            lhsT=v_sb[:, j, 65 * h:65 * h + 65],
                            rhs=pt[:, 512 * h:512 * h + w],
                            start=(j == 0), stop=(j == nkt - 1),
                        )

                pending_oproj = []

                # HAM warmup: run throwaway matmuls while the first DMAs are
                # in flight so the real projections start at the warm clock.
                warm_src = const.tile([128, 512], BF16, tag="warmsrc")
                nc.vector.memset(warm_src[:, :], 1.0)
                for i in range(14):
                    wp = ps_pool.tile([128, 1024], F32, tag="sc", bufs=2,
                                      name=f"warm{i}")
                    nc.tensor.matmul(wp[:, 0:512], lhsT=warm_src[:, 0:128],
                                     rhs=warm_src[:, :], start=True, stop=True)

                emit_qkproj(0)
                emit_vproj(4)
                for qb in range(NQB):
                    # lookahead projections for qb+1, spread through the k-loop
                    # so they soak PE slack instead of stalling the exp stream
                    bg = []
                    if qb + 1 < NQB:
                        for name, dst, bias in (("q", qt_sb, bq_sb),
                                                ("k", kt_sb, bk_sb)):
                            bg.append(lambda n=name, d=dst, b=bias, s=qb + 1:
                                      emit_qkproj_one(n, d, b, s))
                    lo, hi = vproj_done[0], min(4 * (qb + 2), NKT)
                    for st in range(lo, hi):
                        bg.append(lambda st=st: emit_vproj_one(st))
                    vproj_done[0] = hi

                    nkt = 4 * (qb + 1)
                    acc = [ps_pool.tile([65, 512], F32, tag=f"acc{h}",
                                        name=f"acc{h}_{qb}")
                           for h in range(2)]
                    pts = []
                    for j in range(nkt):
                        if j >= 8 and j % 2 == 0 and pending_oproj:
                            # previous q-block's output projection, one tile
                            # per k-tile so its PSUM rotation never stalls PE
                            pending_oproj.pop(0)()
                        elif j >= 2 and bg:
                            bg.pop(0)()
                        r = j - 4 * qb  # >= 0 on the causal diagonal
                        off = 128 * r if r > 0 else 0
                        w = 512 - off   # valid q columns for this k-tile
                        sc = ps_pool.tile([128, 1024], F32, tag="sc", bufs=2,
                                          name=f"sc{qb}_{j}")
                        for h in range(2):
                            hp = slice(64 * h, 64 * h + 64)
                            nc.tensor.matmul(
                                sc[:, 512 * h:512 * h + w],
                                lhsT=kt_sb[hp, j * 128:(j + 1) * 128],
                                rhs=qt_sb[hp, qb * 512 + off:(qb + 1) * 512],
                                start=True, stop=True,
                            )
                        pt = spt.tile([128, 1024], BF16, tag="pt", name=f"pt{qb}_{j}")
                        if r >= 0:
                            # one exp over both heads' [0:w] and [512:512+w]
                            # slices via a strided AP (keeps PSUM banks aligned)
                            def _two(t, w=w):
                                a = t[:, :]
                                return bass.AP(tensor=a.tensor, offset=a.offset,
                                               ap=[a.ap[0], [512, 2], [1, w]])
                            nc.scalar.activation(_two(pt), _two(sc), AF.Exp)
                            m = mask_sb[:, 384:384 + w]
                            for h in range(2):
                                pslc = pt[:, 512 * h:512 * h + w]
                                nc.vector.tensor_mul(pslc, pslc, m)
                        else:
                            nc.scalar.activation(pt[:, :], sc[:, :], AF.Exp)
                        pts.append((j, pt, off, w))
                        if len(pts) >= 7:
                            emit_attnv(acc, pts.pop(0), nkt)
                    while bg:  # anything not soaked up mid-loop
                        bg.pop(0)()
                    while pts:
                        emit_attnv(acc, pts.pop(0), nkt)

                    att = sat.tile([128, 512], BF16, tag="att", name=f"att{qb}")
                    for h in range(2):
                        rc = sdiv.tile([1, 512], F32, tag=f"rc{h}", name=f"rc{h}_{qb}")
                        nc.vector.reciprocal(rc[:, :], acc[h][64:65, :])
                        rbc = sdiv.tile([64, 512], F32, tag=f"rbc{h}",
                                        name=f"rbc{h}_{qb}")
                        nc.gpsimd.partition_broadcast(rbc[:, :], rc[0:1, :])
                        nc.vector.tensor_mul(
                            att[64 * h:64 * h + 64, :], acc[h][0:64, :], rbc[:, :])

                    last = qb == NQB - 1

                    def emit_oproj_one(st, nh, qb=qb, att=att, last=last):
                        op = ps_pool.tile([128, 512], F32, tag="op", bufs=2,
                                          name=f"op{qb}_{st}_{nh}")
                        nc.tensor.matmul(
                            op[:, :],
                            lhsT=att[:, st * 128:(st + 1) * 128],
                            rhs=wo_sb[:, nh * 512:(nh + 1) * 512],
                            start=True, stop=True,
                        )
                        ob = sout.tile([128, 512], F32, tag="ob",
                                       name=f"ob{qb}_{st}_{nh}")
                        if last and (st * 2 + nh) % 2 == 0:
                            # ScalarE is idle in the kernel tail; split the
                            # PSUM drain across both engines
                            nc.scalar.copy(ob[:, :], op[:, :])
                        else:
                            nc.vector.tensor_copy(ob[:, :], op[:, :])
                        nc.sync.dma_start(
                            out=out[qb * 512 + st * 128:
                                    qb * 512 + (st + 1) * 128,
                                    nh * 512:(nh + 1) * 512],
                            in_=ob[:, :])

                    for st in range(4):
                        for nh in range(E // 512):
                            pending_oproj.append(
                                lambda st=st, nh=nh, f=emit_oproj_one: f(st, nh))

                while pending_oproj:
                    pending_oproj.pop(0)()

    nc.compile()
    return nc


def _make_mask_strip():
    k = np.arange(128)[:, None]
    t = np.arange(896)[None, :]
    return (k <= t - 384).astype(np.float32)


def _pack_w(wT):
    # [E, EC] -> [128, NI*EC] with packed[p, it*EC+e] = wT[it*128+p, e]
    E, EC = wT.shape
    return np.ascontiguousarray(
        wT.reshape(E // 128, 128, EC).transpose(1, 0, 2).reshape(128, -1))


def _shard_inputs(x, Wq, bq, Wk, bk, Wv, bv, Wo):
    import ml_dtypes
    bf16 = ml_dtypes.bfloat16
    S, E = x.shape[-2], x.shape[-1]
    xP = np.ascontiguousarray(
        np.asarray(x, np.float32).reshape(S // 512, 512, E // 128, 128)
        .transpose(3, 0, 2, 1)).astype(bf16)
    strip = _make_mask_strip().astype(bf16)
    in_maps = []
    for c in range(N_CORES):
        sl = slice(128 * c, 128 * (c + 1))
        in_maps.append({
            "xP": xP,
            "wqT": _pack_w((np.asarray(Wq, np.float32)[sl, :] / 8.0).T).astype(bf16),
            "wkT": _pack_w(np.asarray(Wk, np.float32)[sl, :].T).astype(bf16),
            "wvT": _pack_w(np.asarray(Wv, np.float32)[sl, :].T).astype(bf16),
            "woT": np.ascontiguousarray(np.asarray(Wo, np.float32)[:, sl].T).astype(bf16),
            "bq": (np.asarray(bq, np.float32)[sl] / 8.0).reshape(128, 1),
            "bk": np.asarray(bk, np.float32)[sl].reshape(128, 1),
            "bv": np.asarray(bv, np.float32)[sl].reshape(1, 128),
            "maskst": strip,
        })
    return in_maps


_NC_CACHE = {}


def kernel(x, Wq, bq, Wk, bk, Wv, bv, Wo, bo):
    x = np.asarray(x)
    B, S, E = x.shape
    if (S, E) not in _NC_CACHE:
        _NC_CACHE[(S, E)] = _build_nc(S=S, E=E)
    nc = _NC_CACHE[(S, E)]

    in_maps = _shard_inputs(x, Wq, bq, Wk, bk, Wv, bv, Wo)
    res = run_bass_kernel_spmd(nc, in_maps, list(range(N_CORES)))

    total = np.zeros((S, E), np.float32)
    for r in res.results:
        total += r["out"]
    total += np.asarray(bo, np.float32)
    return total.reshape(B, S, E).astype(np.float32)
